# revision 10
# baseline (speedup 1.0000x reference)
"""CategoryConsistencyLoss kernel for 8 trn2 NeuronCores.

loss = mean_i clip(||x_i - w_{labels_i}||^2, 1e-12, 1e12)

The reference materializes the full [N, C] squared-distance matrix and then
gathers the label-indexed diagonal entries; only those N entries matter, so
the kernel computes row-wise squared distances directly (O(N*D) instead of
O(N*C*D)).

Key optimizations:
- Rows are sorted by label on the host, so a run of consecutive 128-row
  tiles touches few distinct classes. Tiles are packed into groups whose
  combined distinct-class count fits in 128; one indirect DMA per group
  gathers just those unique weight rows (out-of-bounds padding indices are
  skipped, costing no DMA traffic). This cuts HBM traffic from 33.6MB/core
  (naive per-row gather) to ~18MB/core and needs only ~2 indirect DMAs.
- Unique rows are replicated to per-row alignment with an exact fp32
  0/1-selection matmul on the otherwise idle TensorEngine
  (host-precomputed selection matrices).

Sharding: data-parallel over N across the 8 cores; weightcenters replicated.
Each core returns per-row distances; the host does the final clip + mean
(the row sum is permutation invariant, so the host-side sort needs no undo).
"""

import numpy as np

import concourse.bacc as bacc
import concourse.bass as bass
import concourse.mybir as mybir
import concourse.tile as tile
from concourse import bass_utils

N, C, D = 16384, 1000, 2048
N_CORES = 8
N_LOC = N // N_CORES  # 2048 rows per core
P = 128               # SBUF partitions
T = N_LOC // P        # 16 tiles per core
H = D // 2            # half-tile columns for finer PE->DVE pipelining
PAD_IDX = 1 << 24     # gather index sentinel; > C-1 so the DMA skips it

_nc_cache = {}
LAST_RESULTS = None  # BassKernelResults of the most recent run (for profiling)


def _build(group_of_tile):
    """group_of_tile: tuple of length T mapping tile index -> group index."""
    n_groups = max(group_of_tile) + 1
    nc = bacc.Bacc("TRN2", target_bir_lowering=False, debug=False)
    f32 = mybir.dt.float32
    x_d = nc.dram_tensor("x", [N_LOC, D], f32, kind="ExternalInput")
    uniq_d = nc.dram_tensor(
        "uniq", [P, n_groups], mybir.dt.int32, kind="ExternalInput"
    )
    sel_d = nc.dram_tensor("sel", [T * P, P], f32, kind="ExternalInput")
    w_d = nc.dram_tensor("w", [C, D], f32, kind="ExternalInput")
    out_d = nc.dram_tensor("dist", [P, T], f32, kind="ExternalOutput")

    x_ap = x_d.ap()
    w_ap = w_d.ap()
    sel_ap = sel_d.ap()

    with tile.TileContext(nc) as tc:
        with (
            tc.tile_pool(name="main", bufs=6) as pool,
            tc.tile_pool(name="selp", bufs=1) as selpool,
            tc.tile_pool(name="psum", bufs=4, space="PSUM") as pspool,
            tc.tile_pool(name="small", bufs=1) as spool,
        ):
            # uniq + sel prefetches go on the ACT engine's HW-DGE ring so
            # they are not queued behind the 16.8MB x stream on sync's ring.
            uniq_sb = spool.tile([P, n_groups], mybir.dt.int32)
            nc.scalar.dma_start(out=uniq_sb[:], in_=uniq_d.ap()[:])
            rowsum = spool.tile([P, T], f32)

            sels = []
            for t in range(T):
                sel = selpool.tile([P, P], f32, tag=f"sel{t}")
                nc.scalar.dma_start(out=sel[:], in_=sel_ap[t * P : (t + 1) * P, :])
                sels.append(sel)

            # One gathered unique-rows table per group, resident all kernel.
            # Pre-zeroed because padded (skipped) gather rows keep stale SBUF
            # contents, which the selection matmul multiplies by 0.0 — that
            # must not be NaN.
            wg = []
            for g in range(n_groups):
                wg_g = spool.tile([P, D], f32, tag=f"wg{g}")
                nc.gpsimd.memset(wg_g[:], 0.0)
                nc.gpsimd.indirect_dma_start(
                    out=wg_g[:],
                    out_offset=None,
                    in_=w_ap[:],
                    in_offset=bass.IndirectOffsetOnAxis(
                        ap=uniq_sb[:, g : g + 1], axis=0
                    ),
                    bounds_check=C - 1,
                    oob_is_err=False,
                )
                wg.append(wg_g)

            for t in range(T):
                x_t = pool.tile([P, D], f32, tag="x")
                nc.sync.dma_start(out=x_t[:], in_=x_ap[t * P : (t + 1) * P, :])

                sel = sels[t]
                wg_t = wg[group_of_tile[t]]
                # Expand unique rows to per-row alignment: wexp = sel.T @ wg.
                # 0/1 weights keep fp32 matmul exact. Two PSUM half-tiles per
                # tile so the subtract can drain one half while the PE fills
                # the other.
                for h in range(2):
                    wexp = pspool.tile([P, H], f32, space="PSUM", tag="ps")
                    for q in range(H // 512):
                        nc.tensor.matmul(
                            out=wexp[:, q * 512 : (q + 1) * 512],
                            lhsT=sel[:],
                            rhs=wg_t[:, h * H + q * 512 : h * H + (q + 1) * 512],
                            start=True,
                            stop=True,
                        )
                    xs = x_t[:, h * H : (h + 1) * H]
                    nc.vector.tensor_tensor(
                        out=xs, in0=xs, in1=wexp[:], op=mybir.AluOpType.subtract
                    )
                nc.scalar.activation(
                    out=x_t[:],
                    in_=x_t[:],
                    func=mybir.ActivationFunctionType.Square,
                    accum_out=rowsum[:, t : t + 1],
                )
            nc.sync.dma_start(out=out_d.ap()[:], in_=rowsum[:])
    nc.compile()
    return nc


def _pack_tiles(ls):
    """Greedily pack consecutive tiles into groups of <=128 distinct labels.
    Returns the per-tile group index as a tuple."""
    tile_uniqs = [np.unique(ls[t * P : (t + 1) * P]) for t in range(T)]
    group_of_tile = []
    gidx = 0
    cur_u = tile_uniqs[0]
    group_of_tile.append(0)
    for t in range(1, T):
        u2 = np.union1d(cur_u, tile_uniqs[t])
        if len(u2) <= P:
            cur_u = u2
        else:
            gidx += 1
            cur_u = tile_uniqs[t]
        group_of_tile.append(gidx)
    return tuple(group_of_tile)


def kernel(x, labels, weightcenters):
    global LAST_RESULTS
    x = np.asarray(x, dtype=np.float32)
    labels = np.asarray(labels, dtype=np.int32)
    w = np.ascontiguousarray(np.asarray(weightcenters, dtype=np.float32))

    # Global sort by label so each shard (and tile) spans few classes.
    gorder = np.argsort(labels, kind="stable")
    x_sorted = np.ascontiguousarray(x[gorder])
    l_sorted = labels[gorder]

    # Common packing across cores (the SPMD program is shared): a tile
    # starts a new group wherever ANY core's greedy packing does. This
    # refines every core's own packing, so no group can overflow 128.
    packings = [
        _pack_tiles(l_sorted[c * N_LOC : (c + 1) * N_LOC]) for c in range(N_CORES)
    ]
    starts = {
        t
        for got in packings
        for t in range(1, T)
        if got[t] != got[t - 1]
    }
    common = []
    gidx = 0
    for t in range(T):
        if t in starts:
            gidx += 1
        common.append(gidx)
    common = tuple(common)
    n_groups = common[-1] + 1

    if common not in _nc_cache:
        _nc_cache[common] = _build(common)
    nc = _nc_cache[common]

    in_maps = []
    arange_p = np.arange(P)
    for c in range(N_CORES):
        ls_c = l_sorted[c * N_LOC : (c + 1) * N_LOC]
        uniq = np.full((P, n_groups), PAD_IDX, dtype=np.int32)
        sel = np.zeros((T, P, P), dtype=np.float32)
        for g in range(n_groups):
            tiles = [t for t in range(T) if common[t] == g]
            gu = np.unique(
                ls_c[tiles[0] * P : (tiles[-1] + 1) * P]
            )
            assert len(gu) <= P, "group packing overflow"
            uniq[: len(gu), g] = gu
            for t in tiles:
                e = np.searchsorted(gu, ls_c[t * P : (t + 1) * P])
                sel[t, e, arange_p] = 1.0
        in_maps.append(
            {
                "x": x_sorted[c * N_LOC : (c + 1) * N_LOC],
                "uniq": uniq,
                "sel": sel.reshape(T * P, P),
                "w": w,
            }
        )

    res = bass_utils.run_bass_kernel_spmd(nc, in_maps, core_ids=list(range(N_CORES)))
    LAST_RESULTS = res

    dist = np.concatenate(
        [res.results[c]["dist"].astype(np.float64).T.reshape(-1) for c in range(N_CORES)]
    )
    loss = np.clip(dist, 1e-12, 1e12).sum() / N
    return np.float32(loss)
